# revision 10
# baseline (speedup 1.0000x reference)
"""Trainium2 Bass kernel for a pre-LN causal decoder layer (MHA + SwiGLU).

Sharding: 2-way data parallel over batch x 4-way tensor parallel over heads.
Core c (of 8): batch b=c//4, group rank r=c%4, heads [4r, 4r+4).
Each core computes Q/K/V + causal attention for its 4 heads over its batch's
2048 tokens, a partial ctx @ Wo[rows], then one ReduceScatter(add) over the
4-core group hands each core complete attention output for 512 tokens.
The FFN (SwiGLU, full weights) then runs token-parallel on those 512 tokens;
the host concatenates the 8 output shards.

LayerNorm is folded: matmuls run on raw x^T (host-transposed, bf16) and the
per-token (mean, rstd) fixup is applied to the QKV PSUM with per-partition
scalars; gamma is folded into the weights on the host.
"""

import sys

sys.path.insert(0, "/opt/trn_rl_repo")

import numpy as np
import ml_dtypes

import concourse.bass as bass
import concourse.mybir as mybir
import concourse.tile as tile
from concourse import bacc
from concourse.bass_utils import run_bass_kernel_spmd
from concourse.masks import make_identity

BF16 = ml_dtypes.bfloat16
F32 = mybir.dt.float32
BF = mybir.dt.bfloat16

B, T, C = 2, 2048, 1024
H, HS = 16, 64
HID = 2730
HIDP = 2816  # padded to 22*128
NF = HIDP // 128  # 22
HPC = 4  # heads per core
TLOC = T // 4  # 512 tokens owned post-RS
EPS = 1e-3
NEG = -60.0
RG = [[0, 1, 2, 3], [4, 5, 6, 7]]
NT = T // 128  # 16 token tiles
NJ = T // 512  # 4 t-blocks of 512
NKC = C // 128  # 8 contraction chunks

_cache = {}


def _build(have_bw):
    nc = bacc.Bacc(None, target_bir_lowering=False, debug=False)
    xT = nc.declare_dram_parameter("xT", [C, T], BF, isOutput=False)
    xbf = nc.declare_dram_parameter("xbf", [T, C], BF, isOutput=False)
    xres = nc.declare_dram_parameter("xres", [TLOC, C], F32, isOutput=False)
    wqkv = nc.declare_dram_parameter("wqkv", [C, 768], BF, isOutput=False)
    gws = nc.declare_dram_parameter("gws", [768], F32, isOutput=False)
    if have_bw:
        bw = nc.declare_dram_parameter("bw", [768], F32, isOutput=False)
        bw1 = nc.declare_dram_parameter("bw1", [HIDP], F32, isOutput=False)
        bw2 = nc.declare_dram_parameter("bw2", [HIDP], F32, isOutput=False)
    wo = nc.declare_dram_parameter("wo", [256, C], BF, isOutput=False)
    w1 = nc.declare_dram_parameter("w1", [C, HIDP], BF, isOutput=False)
    w2 = nc.declare_dram_parameter("w2", [C, HIDP], BF, isOutput=False)
    w3 = nc.declare_dram_parameter("w3", [HIDP, C], BF, isOutput=False)
    out = nc.declare_dram_parameter("out", [TLOC, C], F32, isOutput=True)

    rs_in = nc.dram_tensor("rs_in", [T, C], BF)
    rs_out = nc.dram_tensor("rs_out", [TLOC, C], BF)

    def bcast(ap, parts):
        # stride-0 partition broadcast view of a 1-partition AP
        return bass.AP(tensor=ap.tensor, offset=ap.offset,
                       ap=[[0, parts]] + [list(p) for p in ap.ap[-1:]])

    with tile.TileContext(nc) as tc:
        from contextlib import ExitStack
        with ExitStack() as ctx:
            consts = ctx.enter_context(tc.tile_pool(name="consts", bufs=1))
            ident = consts.tile([128, 128], BF)
            make_identity(nc, ident)
            maskc = consts.tile([128, 128], F32)
            nc.gpsimd.memset(maskc, 0.0)
            # keep where col >= row (s <= t), else NEG
            nc.gpsimd.affine_select(
                out=maskc, in_=maskc, compare_op=mybir.AluOpType.is_ge,
                fill=NEG, base=0, pattern=[[1, 128]], channel_multiplier=-1)
            gwsb = consts.tile([128, 768], F32)
            nc.gpsimd.dma_start(out=gwsb, in_=bcast(gws[:], 128))
            epsc = consts.tile([128, 1], F32)
            nc.vector.memset(epsc, EPS)
            if have_bw:
                bwb = consts.tile([128, 768], F32)
                nc.gpsimd.dma_start(out=bwb, in_=bcast(bw[:], 128))
                bw1c = consts.tile([128, NF], F32)
                nc.sync.dma_start(out=bw1c, in_=bw1[:].rearrange("(f p) -> p f", p=128))
                bw2c = consts.tile([128, NF], F32)
                nc.sync.dma_start(out=bw2c, in_=bw2[:].rearrange("(f p) -> p f", p=128))
            wqkv_sb = consts.tile([128, NKC, 768], BF)
            nc.sync.dma_start(out=wqkv_sb, in_=wqkv[:].rearrange("(k p) j -> p k j", p=128))
            wo_sb = consts.tile([128, 2, C], BF)
            nc.sync.dma_start(out=wo_sb, in_=wo[:].rearrange("(k p) j -> p k j", p=128))

            # ---------------- Phase A: stats + fused-LN QKV ----------------
            pA = ctx.enter_context(tc.tile_pool(name="pA", bufs=1))
            q_sb = pA.tile([128, NT, 256], BF)
            k_sb = pA.tile([128, NT, 256], BF)
            v_sb = pA.tile([128, NT, HPC, 65], BF)
            nc.vector.memset(v_sb[:, :, :, 64:65], 1.0)

            with tc.tile_pool(name="xTp", bufs=1) as xTp, \
                 tc.tile_pool(name="stA", bufs=8) as stA, \
                 tc.tile_pool(name="xin", bufs=3) as xin, \
                 tc.tile_pool(name="fixA", bufs=3) as fixA, \
                 tc.tile_pool(name="psA", bufs=2, space="PSUM") as psA:
                xT_sb = xTp.tile([128, NKC, T], BF)
                nc.sync.dma_start(out=xT_sb, in_=xT[:].rearrange("(k p) t -> p k t", p=128))
                for ti in range(NT):
                    xt = xin.tile([128, C], BF)
                    nc.sync.dma_start(out=xt, in_=xbf[ti * 128:(ti + 1) * 128, :])
                    st = stA.tile([128, 2, 6], F32)
                    nc.vector.bn_stats(out=st[:, 0, :], in_=xt[:, 0:512])
                    nc.vector.bn_stats(out=st[:, 1, :], in_=xt[:, 512:1024])
                    mv = stA.tile([128, 2], F32)
                    nc.vector.bn_aggr(out=mv, in_=st)
                    sd = stA.tile([128, 1], F32)
                    nc.scalar.activation(out=sd, in_=mv[:, 1:2],
                                         func=mybir.ActivationFunctionType.Sqrt,
                                         bias=epsc[:, 0:1])
                    rstd = stA.tile([128, 1], F32)
                    nc.vector.reciprocal(rstd, sd)
                    rmu = stA.tile([128, 1], F32)
                    nc.vector.tensor_mul(rmu, mv[:, 0:1], rstd)

                    ps = psA.tile([128, 768], F32)
                    for kc in range(NKC):
                        for n0, n1 in ((0, 512), (512, 768)):
                            nc.tensor.matmul(ps[:, n0:n1],
                                             xT_sb[:, kc, ti * 128:(ti + 1) * 128],
                                             wqkv_sb[:, kc, n0:n1],
                                             start=(kc == 0), stop=(kc == NKC - 1))
                    t1 = fixA.tile([128, 768], F32)
                    nc.vector.tensor_scalar_mul(t1, ps, rstd)
                    t2 = fixA.tile([128, 768], F32)
                    nc.vector.tensor_scalar_mul(t2, gwsb, rmu)
                    if have_bw:
                        nc.vector.tensor_sub(t1, t1, t2)
                        nc.vector.tensor_add(t1[:, 0:512], t1[:, 0:512], bwb[:, 0:512])
                        nc.vector.tensor_copy(q_sb[:, ti, :], t1[:, 0:256])
                        nc.vector.tensor_copy(k_sb[:, ti, :], t1[:, 256:512])
                        nc.vector.tensor_add(
                            v_sb[:, ti, :, 0:64],
                            t1[:, 512:768].rearrange("p (h d) -> p h d", h=HPC),
                            bwb[:, 512:768].rearrange("p (h d) -> p h d", h=HPC))
                    else:
                        nc.vector.tensor_sub(q_sb[:, ti, :], t1[:, 0:256], t2[:, 0:256])
                        nc.vector.tensor_sub(k_sb[:, ti, :], t1[:, 256:512], t2[:, 256:512])
                        nc.vector.tensor_sub(
                            v_sb[:, ti, :, 0:64],
                            t1[:, 512:768].rearrange("p (h d) -> p h d", h=HPC),
                            t2[:, 512:768].rearrange("p (h d) -> p h d", h=HPC))

            # ---------------- Phase B: transpose q,k -> [64, T] per head ----
            pB = ctx.enter_context(tc.tile_pool(name="pB", bufs=1))
            qT = pB.tile([64, HPC, T], BF)
            kT = pB.tile([64, HPC, T], BF)
            with tc.tile_pool(name="psB", bufs=4, space="PSUM") as psB:
                for ti in range(NT):
                    for h in range(HPC):
                        tp = psB.tile([64, 128], BF, tag="tp")
                        nc.tensor.transpose(tp, q_sb[:, ti, h * 64:(h + 1) * 64], ident)
                        nc.vector.tensor_copy(qT[:, h, ti * 128:(ti + 1) * 128], tp)
                        tp2 = psB.tile([64, 128], BF, tag="tp")
                        nc.tensor.transpose(tp2, k_sb[:, ti, h * 64:(h + 1) * 64], ident)
                        nc.vector.tensor_copy(kT[:, h, ti * 128:(ti + 1) * 128], tp2)

            # ---------------- Phase C: causal attention (transposed flash) --
            pC = ctx.enter_context(tc.tile_pool(name="pC", bufs=1))
            ctxT = pC.tile([128, 2, T], BF)  # [256 head-dims, T]
            with tc.tile_pool(name="scps", bufs=2, space="PSUM") as scps, \
                 tc.tile_pool(name="avps", bufs=2, space="PSUM") as avps, \
                 tc.tile_pool(name="band", bufs=4) as bandp, \
                 tc.tile_pool(name="stC", bufs=4) as stC:
                for h in range(HPC):
                    for J in range(NJ):
                        av = avps.tile([65, 512], F32, tag="av")
                        nst = 4 * J + 4  # s-tiles 0..nst-1
                        for i in range(nst):
                            w = i - 4 * J  # >=0 on diagonal tiles
                            off = max(w, 0) * 128
                            ncols = 512 - off
                            sc = scps.tile([128, 512], F32, tag="sc")
                            nc.tensor.matmul(
                                sc[:, off:512],
                                kT[:, h, i * 128:(i + 1) * 128],
                                qT[:, h, J * 512 + off:(J + 1) * 512],
                                start=True, stop=True)
                            if w >= 0:
                                nc.vector.tensor_add(sc[:, off:off + 128],
                                                     sc[:, off:off + 128], maskc)
                            bd = bandp.tile([128, 512], BF, tag="bd")
                            nc.scalar.activation(out=bd[:, off:512], in_=sc[:, off:512],
                                                 func=mybir.ActivationFunctionType.Exp)
                            nc.tensor.matmul(
                                av[:, off:512],
                                v_sb[:, i, h, :],
                                bd[:, off:512],
                                start=(i == 0), stop=(i == nst - 1))
                        # normalize: rows 0..63 are ctx^T, row 64 is denom
                        rrow = stC.tile([1, 512], F32, tag="rr")
                        nc.vector.reciprocal(rrow, av[64:65, :])
                        rb = stC.tile([64, 512], F32, tag="rb")
                        nc.gpsimd.partition_broadcast(rb[:, :], rrow[:, :])
                        po = (h % 2) * 64
                        nc.vector.tensor_mul(
                            ctxT[po:po + 64, h // 2, J * 512:(J + 1) * 512],
                            av[0:64, :], rb)

            # ---------------- Phase D: Wo partial + ReduceScatter -----------
            with tc.tile_pool(name="wops", bufs=2, space="PSUM") as wops, \
                 tc.tile_pool(name="dout", bufs=3) as dout:
                for ti in range(NT):
                    wp = wops.tile([128, C], F32, tag="wp")
                    for dc in range(2):
                        for n0, n1 in ((0, 512), (512, 1024)):
                            nc.tensor.matmul(wp[:, n0:n1],
                                             ctxT[:, dc, ti * 128:(ti + 1) * 128],
                                             wo_sb[:, dc, n0:n1],
                                             start=(dc == 0), stop=(dc == 1))
                    ab = dout.tile([128, C], BF, tag="ab")
                    nc.vector.tensor_copy(ab, wp)
                    nc.sync.dma_start(out=rs_in[ti * 128:(ti + 1) * 128, :], in_=ab)
            nc.gpsimd.collective_compute(
                "ReduceScatter", mybir.AluOpType.add, replica_groups=RG,
                ins=[rs_in[:]], outs=[rs_out[:]])

            # ---------------- Phase E: residual + LN2 + SwiGLU FFN ----------
            pE = ctx.enter_context(tc.tile_pool(name="pE", bufs=1))
            out1 = pE.tile([128, 4, C], F32)
            hn2T = pE.tile([128, NKC, TLOC], BF)
            g_sb = pE.tile([128, NF, TLOC], BF)
            with tc.tile_pool(name="ein", bufs=3) as ein, \
                 tc.tile_pool(name="stE", bufs=6) as stE, \
                 tc.tile_pool(name="psE", bufs=4, space="PSUM") as psE:
                for tt in range(4):
                    rst = ein.tile([128, C], BF, tag="rst")
                    nc.sync.dma_start(out=rst, in_=rs_out[tt * 128:(tt + 1) * 128, :])
                    xrt = ein.tile([128, C], F32, tag="xrt")
                    nc.sync.dma_start(out=xrt, in_=xres[tt * 128:(tt + 1) * 128, :])
                    o1 = out1[:, tt, :]
                    nc.vector.tensor_add(o1, xrt, rst)
                    st = stE.tile([128, 2, 6], F32, tag="st")
                    nc.vector.bn_stats(out=st[:, 0, :], in_=o1[:, 0:512])
                    nc.vector.bn_stats(out=st[:, 1, :], in_=o1[:, 512:1024])
                    mv = stE.tile([128, 2], F32, tag="mv")
                    nc.vector.bn_aggr(out=mv, in_=st)
                    sd = stE.tile([128, 1], F32, tag="sd")
                    nc.scalar.activation(out=sd, in_=mv[:, 1:2],
                                         func=mybir.ActivationFunctionType.Sqrt,
                                         bias=epsc[:, 0:1])
                    rstd = stE.tile([128, 1], F32, tag="rstd")
                    nc.vector.reciprocal(rstd, sd)
                    rmu = stE.tile([128, 1], F32, tag="rmu")
                    nc.vector.tensor_mul(rmu, mv[:, 0:1], rstd)
                    hn2 = stE.tile([128, C], BF, tag="hn2")
                    nc.vector.tensor_scalar(hn2, o1, rstd, rmu,
                                            mybir.AluOpType.mult,
                                            mybir.AluOpType.subtract)
                    for kc in range(NKC):
                        tp = psE.tile([128, 128], BF, tag="tpE")
                        nc.tensor.transpose(tp, hn2[:, kc * 128:(kc + 1) * 128], ident)
                        nc.vector.tensor_copy(hn2T[:, kc, tt * 128:(tt + 1) * 128], tp)

            with tc.tile_pool(name="wstream", bufs=3) as wstream, \
                 tc.tile_pool(name="gtmp", bufs=3) as gtmp, \
                 tc.tile_pool(name="psG", bufs=4, space="PSUM") as psG:
                for fi in range(NF):
                    w1t = wstream.tile([128, NKC, 128], BF, tag="w1t")
                    nc.sync.dma_start(
                        out=w1t,
                        in_=w1[:, fi * 128:(fi + 1) * 128].rearrange("(k p) f -> p k f", p=128))
                    w2t = wstream.tile([128, NKC, 128], BF, tag="w2t")
                    nc.sync.dma_start(
                        out=w2t,
                        in_=w2[:, fi * 128:(fi + 1) * 128].rearrange("(k p) f -> p k f", p=128))
                    g1 = psG.tile([128, TLOC], F32, tag="g1")
                    for kc in range(NKC):
                        nc.tensor.matmul(g1, w1t[:, kc, :], hn2T[:, kc, :],
                                         start=(kc == 0), stop=(kc == NKC - 1))
                    sil = gtmp.tile([128, TLOC], BF, tag="sil")
                    nc.scalar.activation(out=sil, in_=g1,
                                         func=mybir.ActivationFunctionType.Silu,
                                         bias=(bw1c[:, fi:fi + 1] if have_bw else 0.0))
                    g2 = psG.tile([128, TLOC], F32, tag="g2")
                    for kc in range(NKC):
                        nc.tensor.matmul(g2, w2t[:, kc, :], hn2T[:, kc, :],
                                         start=(kc == 0), stop=(kc == NKC - 1))
                    if have_bw:
                        nc.vector.tensor_scalar_add(g2, g2, bw2c[:, fi:fi + 1])
                    nc.vector.tensor_mul(g_sb[:, fi, :], sil, g2)

            with tc.tile_pool(name="w3s", bufs=3) as w3s, \
                 tc.tile_pool(name="oout", bufs=3) as oout, \
                 tc.tile_pool(name="psW3", bufs=1, space="PSUM") as psW3:
                for half in range(2):
                    acc = psW3.tile([128, 4, 512], F32, tag="acc")
                    for fi in range(NF):
                        w3t = w3s.tile([128, 512], BF, tag="w3t")
                        nc.sync.dma_start(
                            out=w3t,
                            in_=w3[fi * 128:(fi + 1) * 128, half * 512:(half + 1) * 512])
                        for tt in range(4):
                            nc.tensor.matmul(acc[:, tt, :],
                                             g_sb[:, fi, tt * 128:(tt + 1) * 128], w3t,
                                             start=(fi == 0), stop=(fi == NF - 1))
                    for tt in range(4):
                        ot = oout.tile([128, 512], F32, tag="ot")
                        nc.vector.tensor_add(ot, acc[:, tt, :],
                                             out1[:, tt, half * 512:(half + 1) * 512])
                        nc.sync.dma_start(
                            out=out[tt * 128:(tt + 1) * 128, half * 512:(half + 1) * 512],
                            in_=ot)
    nc.compile()
    return nc


def _prep(x, Wq, Wk, Wv, Wo, W1, W2, W3, gamma, beta):
    f32 = np.float32
    scale = f32(1.0 / np.sqrt(HS))
    gcol = gamma.astype(f32)[:, None]
    in_maps = []
    for c in range(8):
        b, r = c // 4, c % 4
        hh = slice(r * HPC, (r + 1) * HPC)
        # per-head [C, HS] blocks -> [C, 256] column groups
        qc = Wq[hh].transpose(1, 0, 2).reshape(C, 256).astype(f32) * scale
        kc = Wk[hh].transpose(1, 0, 2).reshape(C, 256).astype(f32)
        vc = Wv[hh].transpose(1, 0, 2).reshape(C, 256).astype(f32)
        wcat = np.concatenate([qc, kc, vc], axis=1)  # [C, 768], scale folded in q
        wq_g = gcol * wcat
        gws = wq_g.sum(axis=0).astype(f32)
        bw = (beta.astype(f32) @ wcat).astype(f32)
        w1p = np.zeros((C, HIDP), f32)
        w1p[:, :HID] = W1
        w2p = np.zeros((C, HIDP), f32)
        w2p[:, :HID] = W2
        w3p = np.zeros((HIDP, C), f32)
        w3p[:HID, :] = W3
        bw1 = (beta.astype(f32) @ w1p).astype(f32)
        bw2 = (beta.astype(f32) @ w2p).astype(f32)
        xb = x[b].astype(f32)
        m = {
            "xT": np.ascontiguousarray(xb.T).astype(BF16),
            "xbf": xb.astype(BF16),
            "xres": np.ascontiguousarray(xb[r * TLOC:(r + 1) * TLOC]),
            "wqkv": wq_g.astype(BF16),
            "gws": gws,
            "wo": np.ascontiguousarray(Wo[r * 256:(r + 1) * 256, :]).astype(BF16),
            "w1": (gcol * w1p).astype(BF16),
            "w2": (gcol * w2p).astype(BF16),
            "w3": w3p.astype(BF16),
        }
        have_bw = bool(np.any(beta != 0))
        if have_bw:
            m["bw"] = bw
            m["bw1"] = bw1
            m["bw2"] = bw2
        in_maps.append(m)
    return in_maps, have_bw


def kernel(x, Wq, Wk, Wv, Wo, W1, W2, W3, gamma, beta, _bench=None):
    x = np.asarray(x)
    in_maps, have_bw = _prep(np.asarray(x), np.asarray(Wq), np.asarray(Wk),
                             np.asarray(Wv), np.asarray(Wo), np.asarray(W1),
                             np.asarray(W2), np.asarray(W3),
                             np.asarray(gamma), np.asarray(beta))
    key = ("k", have_bw)
    if key not in _cache:
        _cache[key] = _build(have_bw)
    nc = _cache[key]
    kw = dict(_bench) if _bench else {}
    res = run_bass_kernel_spmd(nc, in_maps, list(range(8)), **kw)
    outf = np.empty((B, T, C), np.float32)
    for c in range(8):
        b, r = c // 4, c % 4
        outf[b, r * TLOC:(r + 1) * TLOC] = res.results[c]["out"]
    if _bench is not None:
        kernel.last_results = res
    return outf
